# revision 6
# baseline (speedup 1.0000x reference)
"""Trainium2 Bass kernel for causal multi-head attention with QKV projections.

Problem: x [2, 4096, 1024], Wq/Wk/Wv [1024, 1024] (nn.Linear, y = x @ W.T),
16 heads x 64 dim, causal softmax attention, output [2, 4096, 1024] fp32.

Sharding: 8 cores; core c handles batch b = c // 4 and heads
[4*(c%4), 4*(c%4)+4)  (4 heads = 256 channels per core). No cross-core comm.

Per-core device pipeline (all matmul streams in bf16, fp32 PSUM accumulate):
  1. Projections: Q^T/K^T/V^T [256, T] = W*.T-slices^T @ x^T, streamed over
     8 contraction tiles of 128 (bf16, N=512 moving). The 1/sqrt(D) score
     scale is folded into Wq on the host.
  2. V^T -> V via PE transposes (fp32r path); V stored bf16 as
     [128k, 4h, 32j, 66] with a ones column (64, for softmax sums) and a
     zero pad column (65, keeps matmul dst dims even).
  3. Attention per (head-pair, 512-query group): for each 128-key tile j,
     two concurrent QK matmuls (head A at partitions 0-63, head B at 64-127
     -> distinct PE row groups) write S^T[k, q] for both heads into one
     PSUM tile; one exp on ACT (PSUM -> SBUF bf16, 1024 wide); causal-zero
     of diagonal tiles via gpsimd affine_select; then per head accumulate
     O^T[66, 512] += V_ext_j^T E_j^T in PSUM.
  4. Finalize per head: O^T -> SBUF, PE-transpose 128-query blocks to
     [128, 66]; row 64 holds the softmax denominators -> reciprocal +
     per-partition scale; one batched DMA per (head, group) to DRAM.
"""

import os
import sys

import numpy as np

try:  # the axon site normally provides concourse; fall back to the repo copy
    import concourse  # noqa: F401
except ImportError:  # pragma: no cover
    sys.path.insert(0, "/opt/trn_rl_repo")

from contextlib import ExitStack

import ml_dtypes
import concourse.bass as bass  # noqa: F401  (AP helpers)
import concourse.tile as tile
from concourse import bacc, bass_utils, mybir
from concourse.masks import make_identity

FP = mybir.dt.float32
FR = mybir.dt.float32r
BF = mybir.dt.bfloat16
AF = mybir.ActivationFunctionType

B, T_FULL, C = 2, 4096, 1024
H, D = 16, 64
N_CORES = 8
HPC = 4            # heads per core
CPC = HPC * D      # channels per core (256)
QG = 512           # query-group width
VC = D + 2         # V columns incl ones + pad (66)

_CACHE = {}


def _emit(ctx, tc, t):
    nc = tc.nc
    nkt = t // 128       # key tiles
    nqg = t // QG        # query groups
    ntc = t // 512       # projection t-chunks

    xT = nc.dram_tensor("xT", [C, t], BF, kind="ExternalInput").ap()
    wqT = nc.dram_tensor("wqT", [C, CPC], BF, kind="ExternalInput").ap()
    wkT = nc.dram_tensor("wkT", [C, CPC], BF, kind="ExternalInput").ap()
    wvT = nc.dram_tensor("wvT", [C, CPC], BF, kind="ExternalInput").ap()
    out = nc.dram_tensor("out", [t, CPC], FP, kind="ExternalOutput").ap()

    # ---------------- persistent SBUF ----------------
    big = ctx.enter_context(tc.tile_pool(name="big", bufs=1))
    ident_fp = big.tile([128, 128], FP, tag="ident_fp")
    make_identity(nc, ident_fp)
    ident = big.tile([128, 128], FR, tag="ident")
    nc.vector.tensor_copy(ident, ident_fp)

    # Q^T / K^T head-pair tiles: partition p -> head hp*2 + p//64, dim p%64
    qt = [big.tile([128, t], BF, tag=f"qt{hp}", name=f"qt{hp}") for hp in range(HPC // 2)]
    kt = [big.tile([128, t], BF, tag=f"kt{hp}", name=f"kt{hp}") for hp in range(HPC // 2)]
    # V (bf16) with ones column (64) and zero pad (65)
    v_sb = big.tile([128, HPC, nkt, VC], BF, tag="v_sb")

    # ---------------- phase 1: projections ----------------
    with ExitStack() as p1:
        wp = p1.enter_context(tc.tile_pool(name="wp", bufs=1))
        xtp = p1.enter_context(tc.tile_pool(name="xtp", bufs=2))
        ppsum = p1.enter_context(tc.tile_pool(name="ppsum", bufs=1, space="PSUM"))
        vtp = p1.enter_context(tc.tile_pool(name="vtp", bufs=2, space="PSUM"))

        stage = wp.tile([128, HPC, nkt, 2], FP, tag="stage")
        nc.vector.memset(stage[:, :, :, 0:1], 1.0)
        nc.vector.memset(stage[:, :, :, 1:2], 0.0)
        nc.vector.tensor_copy(v_sb[:, :, :, D : D + 2], stage)

        w_sb = {}
        for name, dram in (("wq", wqT), ("wk", wkT), ("wv", wvT)):
            w_tile = wp.tile([128, 8, CPC], BF, tag=name)
            src = dram.rearrange("(k p) m -> p k m", p=128)
            for k in range(8):
                nc.sync.dma_start(w_tile[:, k, :], src[:, k, :])
            w_sb[name] = w_tile
        vt = [wp.tile([128, t], FR, tag=f"vt{hp}", name=f"vt{hp}") for hp in range(HPC // 2)]

        xT_r = xT.rearrange("(k p) t -> p k t", p=128)
        dests = (
            [(w_sb["wq"], qt[m]) for m in range(2)]
            + [(w_sb["wk"], kt[m]) for m in range(2)]
            + [(w_sb["wv"], vt[m]) for m in range(2)]
        )
        for ch in range(ntc):
            tsl = slice(ch * 512, (ch + 1) * 512)
            x_tile = xtp.tile([128, 8, 512], BF, tag="x")
            for k in range(8):
                nc.sync.dma_start(x_tile[:, k, :], xT_r[:, k, tsl])
            psums = []
            for i, (w_tile, dst) in enumerate(dests):
                m = i % 2
                pp = ppsum.tile([128, 512], FP, tag=f"pp{i}")
                for k in range(8):
                    nc.tensor.matmul(
                        pp,
                        lhsT=w_tile[:, k, m * 128 : (m + 1) * 128],
                        rhs=x_tile[:, k, :],
                        start=(k == 0),
                        stop=(k == 7),
                    )
                psums.append(pp)
            for (w_tile, dst), pp in zip(dests, psums):
                nc.any.tensor_copy(dst[:, tsl], pp)

        # ---------------- phase 2: V^T -> V (PE transposes) ----------------
        for h in range(HPC):
            hp, po = h // 2, 64 * (h % 2)
            for j in range(nkt):
                pt = vtp.tile([128, D], FR, tag="pt")
                nc.tensor.transpose(
                    pt,
                    vt[hp][po : po + 64, j * 128 : (j + 1) * 128],
                    ident[po : po + 64, po : po + 64],
                )
                nc.any.tensor_copy(v_sb[:, h, j, 0:D], pt)

    # ---------------- phase 3: attention ----------------
    with ExitStack() as p3:
        spsum = p3.enter_context(tc.tile_pool(name="spsum", bufs=3, space="PSUM"))
        opsum = p3.enter_context(tc.tile_pool(name="opsum", bufs=2, space="PSUM"))
        ep = p3.enter_context(tc.tile_pool(name="ep", bufs=4))
        fin = p3.enter_context(tc.tile_pool(name="fin", bufs=3))

        for hp in range(HPC // 2):
            hA, hB = 2 * hp, 2 * hp + 1
            for g in range(nqg):
                qsl = slice(g * QG, (g + 1) * QG)
                jmax = (g + 1) * (QG // 128) - 1
                ots = {}
                for hh in (0, 1):
                    o_t = opsum.tile([VC, QG], FP, tag="ot", name=f"ot{hh}")
                    ots[hh] = o_t

                def emit_qk(j):
                    """Two concurrent QK matmuls (row groups 0-63 / 64-127) +
                    one exp over both heads, + causal zeroing on diagonals."""
                    sp = spsum.tile([128, 2, QG], FP, tag="sp")
                    e = ep.tile([128, 2, QG], BF, tag="e")
                    for hh in (0, 1):
                        po = 64 * hh
                        nc.tensor.matmul(
                            sp[:, hh, :],
                            lhsT=kt[hp][po : po + 64, j * 128 : (j + 1) * 128],
                            rhs=qt[hp][po : po + 64, qsl],
                            start=True,
                            stop=True,
                        )
                    nc.scalar.activation(
                        out=e.rearrange("p a b -> p (a b)"),
                        in_=sp.rearrange("p a b -> p (a b)"),
                        func=AF.Exp,
                    )
                    m = j - (g * QG) // 128
                    if m >= 0:  # diagonal tile: causal zeroing (both heads)
                        for hh in (0, 1):
                            blk = e[:, hh, :]
                            nc.gpsimd.affine_select(
                                out=blk,
                                in_=blk,
                                compare_op=mybir.AluOpType.is_ge,
                                fill=0.0,
                                base=-m * 128,
                                pattern=[[1, QG]],
                                channel_multiplier=-1,
                            )
                    return e

                def emit_av(j, e):
                    for hh, h in ((0, hA), (1, hB)):
                        nc.tensor.matmul(
                            ots[hh],
                            lhsT=v_sb[:, h, j, :],
                            rhs=e[:, hh, :],
                            start=(j == 0),
                            stop=(j == jmax),
                        )

                pending = []
                for j in range(jmax + 1):
                    pending.append((j, emit_qk(j)))
                    if len(pending) > 2:
                        emit_av(*pending.pop(0))
                for item in pending:
                    emit_av(*item)

                # finalize: transpose + normalize + one batched store per head
                for hh, h in ((0, hA), (1, hB)):
                    osb = fin.tile([VC, QG], FR, tag="osb")
                    nc.any.tensor_copy(osb, ots[hh])
                    o_sb = fin.tile([128, QG // 128, D], FP, tag="o_sb")
                    for qb in range(QG // 128):
                        ft = opsum.tile([128, VC], FR, tag="ot", name=f"ft{qb}")
                        nc.tensor.transpose(
                            ft,
                            osb[:, qb * 128 : (qb + 1) * 128],
                            ident[0:VC, 0:VC],
                        )
                        recip = fin.tile([128, 1], FP, tag="recip")
                        nc.vector.reciprocal(recip, ft[:, D : D + 1])
                        nc.vector.tensor_scalar_mul(o_sb[:, qb, :], ft[:, 0:D], recip)
                    nc.sync.dma_start(
                        out[qsl, h * D : (h + 1) * D].rearrange(
                            "(qb p) d -> p qb d", p=128
                        ),
                        o_sb,
                    )


def build_program(t=T_FULL):
    if t in _CACHE:
        return _CACHE[t]
    nc = bacc.Bacc("TRN2", target_bir_lowering=False, debug=False)
    with tile.TileContext(nc) as tc:
        with ExitStack() as ctx:
            _emit(ctx, tc, t)
    nc.compile()
    _CACHE[t] = nc
    return nc


def make_in_maps(x, Wq, Wk, Wv):
    """Host-side shard: returns the 8 per-core input maps."""
    x = np.asarray(x, dtype=np.float32)
    Wq = np.asarray(Wq, dtype=np.float32)
    Wk = np.asarray(Wk, dtype=np.float32)
    Wv = np.asarray(Wv, dtype=np.float32)
    scale = np.float32(D ** -0.5)
    bf = ml_dtypes.bfloat16
    xT = np.ascontiguousarray(x.transpose(0, 2, 1)).astype(bf)  # [B, C, T]
    in_maps = []
    for core in range(N_CORES):
        b, hg = divmod(core, N_CORES // B)
        sl = slice(hg * CPC, (hg + 1) * CPC)
        in_maps.append(
            {
                "xT": xT[b],
                "wqT": (np.ascontiguousarray(Wq[sl].T) * scale).astype(bf),
                "wkT": np.ascontiguousarray(Wk[sl].T).astype(bf),
                "wvT": np.ascontiguousarray(Wv[sl].T).astype(bf),
            }
        )
    return in_maps


LAST_RESULTS = None


def kernel(x, Wq, Wk, Wv, _trace=False):
    global LAST_RESULTS
    in_maps = make_in_maps(x, Wq, Wk, Wv)
    nc = build_program(T_FULL)
    res = bass_utils.run_bass_kernel_spmd(
        nc, in_maps, core_ids=list(range(N_CORES)), trace=_trace
    )
    LAST_RESULTS = res
    full = np.empty((B, T_FULL, C), np.float32)
    for core in range(N_CORES):
        b, hg = divmod(core, N_CORES // B)
        full[b, :, hg * CPC : (hg + 1) * CPC] = res.results[core]["out"]
    return full


# revision 10
# speedup vs baseline: 1.0100x; 1.0100x over previous
"""Trainium2 Bass kernel for causal multi-head attention with QKV projections.

Problem: x [2, 4096, 1024], Wq/Wk/Wv [1024, 1024] (nn.Linear, y = x @ W.T),
16 heads x 64 dim, causal softmax attention, output [2, 4096, 1024] fp32.

Sharding: 8 cores; core c handles batch b = c // 4 and heads
[4*(c%4), 4*(c%4)+4)  (4 heads = 256 channels per core). No cross-core comm.

Per-core device pipeline (all matmul streams in bf16, fp32 PSUM accumulate):
  1. Projections: Q^T/K^T/V^T [256, T] = W*.T-slices^T @ x^T, streamed over
     8 contraction tiles of 128 (bf16, N=512 moving). The 1/sqrt(D) score
     scale is folded into Wq on the host.
  2. V^T -> V via PE transposes (fp32r path); V stored bf16 as
     [128k, 4h, 32j, 66] with a ones column (64, for softmax sums) and a
     zero pad column (65, keeps matmul dst dims even).
  3. Attention per (head-pair, 512-query group): for each 128-key tile j,
     two concurrent QK matmuls (head A at partitions 0-63, head B at 64-127
     -> distinct PE row groups) write S^T[k, q] for both heads into one
     PSUM tile; one exp on ACT (PSUM -> SBUF bf16, 1024 wide); causal-zero
     of diagonal tiles via gpsimd affine_select; then per head accumulate
     O^T[66, 512] += V_ext_j^T E_j^T in PSUM.
  4. Finalize per head: O^T -> SBUF, PE-transpose 128-query blocks to
     [128, 66]; row 64 holds the softmax denominators -> reciprocal +
     per-partition scale; one batched DMA per (head, group) to DRAM.
"""

import os
import sys

import numpy as np

try:  # the axon site normally provides concourse; fall back to the repo copy
    import concourse  # noqa: F401
except ImportError:  # pragma: no cover
    sys.path.insert(0, "/opt/trn_rl_repo")

from contextlib import ExitStack

import ml_dtypes
import concourse.bass as bass  # noqa: F401  (AP helpers)
import concourse.tile as tile
from concourse import bacc, bass_utils, mybir
from concourse.masks import make_identity

FP = mybir.dt.float32
FR = mybir.dt.float32r
BF = mybir.dt.bfloat16
AF = mybir.ActivationFunctionType

B, T_FULL, C = 2, 4096, 1024
H, D = 16, 64
N_CORES = 8
HPC = 4            # heads per core
CPC = HPC * D      # channels per core (256)
QG = 512           # query-group width
VC = D + 2         # V columns incl ones + pad (66)

_CACHE = {}


def _emit(ctx, tc, t):
    nc = tc.nc
    nkt = t // 128       # key tiles
    nqg = t // QG        # query groups
    ntc = t // 512       # projection t-chunks

    xT = nc.dram_tensor("xT", [C, t], BF, kind="ExternalInput").ap()
    wqT = nc.dram_tensor("wqT", [C, CPC], BF, kind="ExternalInput").ap()
    wkT = nc.dram_tensor("wkT", [C, CPC], BF, kind="ExternalInput").ap()
    wvT = nc.dram_tensor("wvT", [C, CPC], BF, kind="ExternalInput").ap()
    out = nc.dram_tensor("out", [t, CPC], FP, kind="ExternalOutput").ap()

    # ---------------- persistent SBUF ----------------
    big = ctx.enter_context(tc.tile_pool(name="big", bufs=1))
    ident_fp = big.tile([128, 128], FP, tag="ident_fp")
    make_identity(nc, ident_fp)
    ident = big.tile([128, 128], FR, tag="ident")
    nc.vector.tensor_copy(ident, ident_fp)

    # Q^T / K^T head-pair tiles: partition p -> head hp*2 + p//64, dim p%64
    qt = [big.tile([128, t], BF, tag=f"qt{hp}", name=f"qt{hp}") for hp in range(HPC // 2)]
    kt = [big.tile([128, t], BF, tag=f"kt{hp}", name=f"kt{hp}") for hp in range(HPC // 2)]
    # V (bf16) with ones column (64) and zero pad (65)
    v_sb = big.tile([128, HPC, nkt, VC], BF, tag="v_sb")

    # ---------------- phase 1: projections ----------------
    with ExitStack() as p1:
        wp = p1.enter_context(tc.tile_pool(name="wp", bufs=1))
        xtp = p1.enter_context(tc.tile_pool(name="xtp", bufs=2))
        ppsum = p1.enter_context(tc.tile_pool(name="ppsum", bufs=1, space="PSUM"))
        vtp = p1.enter_context(tc.tile_pool(name="vtp", bufs=2, space="PSUM"))

        stage = wp.tile([128, HPC, nkt, 2], FP, tag="stage")
        nc.vector.memset(stage[:, :, :, 0:1], 1.0)
        nc.vector.memset(stage[:, :, :, 1:2], 0.0)
        nc.vector.tensor_copy(v_sb[:, :, :, D : D + 2], stage)

        w_sb = {}
        for name, dram in (("wq", wqT), ("wk", wkT), ("wv", wvT)):
            w_tile = wp.tile([128, 8, CPC], BF, tag=name)
            src = dram.rearrange("(k p) m -> p k m", p=128)
            for k in range(8):
                nc.sync.dma_start(w_tile[:, k, :], src[:, k, :])
            w_sb[name] = w_tile
        vt = [wp.tile([128, t], FR, tag=f"vt{hp}", name=f"vt{hp}") for hp in range(HPC // 2)]

        xT_r = xT.rearrange("(k p) t -> p k t", p=128)
        dests = (
            [(w_sb["wq"], qt[m]) for m in range(2)]
            + [(w_sb["wk"], kt[m]) for m in range(2)]
            + [(w_sb["wv"], vt[m]) for m in range(2)]
        )
        pn = 512
        for ch in range(ntc):
            tsl = slice(ch * pn, (ch + 1) * pn)
            x_tile = xtp.tile([128, 8, pn], BF, tag="x")
            for k in range(8):
                nc.sync.dma_start(x_tile[:, k, :], xT_r[:, k, tsl])
            psums = []
            for i, (w_tile, dst) in enumerate(dests):
                m = i % 2
                pp = ppsum.tile([128, pn], FP, tag=f"pp{i}")
                for k in range(8):
                    nc.tensor.matmul(
                        pp,
                        lhsT=w_tile[:, k, m * 128 : (m + 1) * 128],
                        rhs=x_tile[:, k, :],
                        start=(k == 0),
                        stop=(k == 7),
                    )
                psums.append(pp)
            for (w_tile, dst), pp in zip(dests, psums):
                nc.vector.tensor_copy(dst[:, tsl], pp)

        # ---------------- phase 2: V^T -> V (PE transposes) ----------------
        for h in range(HPC):
            hp, po = h // 2, 64 * (h % 2)
            for j in range(nkt):
                pt = vtp.tile([128, D], FR, tag="pt")
                nc.tensor.transpose(
                    pt,
                    vt[hp][po : po + 64, j * 128 : (j + 1) * 128],
                    ident[po : po + 64, po : po + 64],
                )
                nc.vector.tensor_copy(v_sb[:, h, j, 0:D], pt)

    # ---------------- phase 3: attention ----------------
    with ExitStack() as p3:
        spsum = p3.enter_context(tc.tile_pool(name="spsum", bufs=3, space="PSUM"))
        opsum = p3.enter_context(tc.tile_pool(name="opsum", bufs=2, space="PSUM"))
        ep = p3.enter_context(tc.tile_pool(name="ep", bufs=4))
        fin = p3.enter_context(tc.tile_pool(name="fin", bufs=3))

        def make_group(hp, g):
            """Closure bundle for one (head-pair, query-group)."""
            qsl = slice(g * QG, (g + 1) * QG)
            jmax = (g + 1) * (QG // 128) - 1
            st = {"ots": None}

            def emit_qk(j):
                sp = spsum.tile([128, 2, QG], FP, tag="sp")
                e = ep.tile([128, 2, QG], BF, tag="e")
                for hh in (0, 1):
                    po = 64 * hh
                    nc.tensor.matmul(
                        sp[:, hh, :],
                        lhsT=kt[hp][po : po + 64, j * 128 : (j + 1) * 128],
                        rhs=qt[hp][po : po + 64, qsl],
                        start=True,
                        stop=True,
                    )
                nc.scalar.activation(
                    out=e.rearrange("p a b -> p (a b)"),
                    in_=sp.rearrange("p a b -> p (a b)"),
                    func=AF.Exp,
                )
                m = j - (g * QG) // 128
                if m >= 0:  # diagonal tile: causal zeroing (both heads)
                    for hh in (0, 1):
                        blk = e[:, hh, :]
                        nc.gpsimd.affine_select(
                            out=blk,
                            in_=blk,
                            compare_op=mybir.AluOpType.is_ge,
                            fill=0.0,
                            base=-m * 128,
                            pattern=[[1, QG]],
                            channel_multiplier=-1,
                        )
                return e

            def emit_av(j, e):
                if st["ots"] is None:
                    st["ots"] = [
                        opsum.tile([VC, QG], FP, tag="ot", name=f"ot{hp}_{g}_{hh}")
                        for hh in (0, 1)
                    ]
                for hh, h in ((0, 2 * hp), (1, 2 * hp + 1)):
                    nc.tensor.matmul(
                        st["ots"][hh],
                        lhsT=v_sb[:, h, j, :],
                        rhs=e[:, hh, :],
                        start=(j == 0),
                        stop=(j == jmax),
                    )
                if j == jmax:
                    finalize()

            def finalize():
                for hh, h in ((0, 2 * hp), (1, 2 * hp + 1)):
                    osb = fin.tile([VC, QG], FR, tag="osb")
                    nc.vector.tensor_copy(osb, st["ots"][hh])
                    o_sb = fin.tile([128, QG // 128, D], FP, tag="o_sb")
                    for qb in range(QG // 128):
                        ft = opsum.tile([128, VC], FR, tag="ot", name=f"ft{qb}")
                        nc.tensor.transpose(
                            ft,
                            osb[:, qb * 128 : (qb + 1) * 128],
                            ident[0:VC, 0:VC],
                        )
                        recip = fin.tile([128, 1], FP, tag="recip")
                        nc.vector.reciprocal(recip, ft[:, D : D + 1])
                        nc.vector.tensor_scalar_mul(o_sb[:, qb, :], ft[:, 0:D], recip)
                    nc.sync.dma_start(
                        out[qsl, h * D : (h + 1) * D].rearrange(
                            "(qb p) d -> p qb d", p=128
                        ),
                        o_sb,
                    )

            return emit_qk, emit_av, jmax

        # One continuous software pipeline over every (head-pair, group, key
        # tile): QK runs 3 tiles ahead of AV so exp latency stays hidden and
        # the PE never sees a multi-us idle window (HAM stays warm).
        groups = {}
        stream = []
        for hp in range(HPC // 2):
            for g in range(nqg):
                groups[(hp, g)] = make_group(hp, g)
                jmax = groups[(hp, g)][2]
                stream.extend((hp, g, j) for j in range(jmax + 1))
        pending = []
        for hp, g, j in stream:
            emit_qk, emit_av, _ = groups[(hp, g)]
            pending.append((emit_av, j, emit_qk(j)))
            if len(pending) > 2:
                av, jj, e = pending.pop(0)
                av(jj, e)
        for av, jj, e in pending:
            av(jj, e)

def build_program(t=T_FULL):
    if t in _CACHE:
        return _CACHE[t]
    nc = bacc.Bacc("TRN2", target_bir_lowering=False, debug=False)
    with tile.TileContext(nc) as tc:
        with ExitStack() as ctx:
            _emit(ctx, tc, t)
    nc.compile()
    _CACHE[t] = nc
    return nc


def make_in_maps(x, Wq, Wk, Wv):
    """Host-side shard: returns the 8 per-core input maps."""
    x = np.asarray(x, dtype=np.float32)
    Wq = np.asarray(Wq, dtype=np.float32)
    Wk = np.asarray(Wk, dtype=np.float32)
    Wv = np.asarray(Wv, dtype=np.float32)
    scale = np.float32(D ** -0.5)
    bf = ml_dtypes.bfloat16
    xT = np.ascontiguousarray(x.transpose(0, 2, 1)).astype(bf)  # [B, C, T]
    in_maps = []
    for core in range(N_CORES):
        b, hg = divmod(core, N_CORES // B)
        sl = slice(hg * CPC, (hg + 1) * CPC)
        in_maps.append(
            {
                "xT": xT[b],
                "wqT": (np.ascontiguousarray(Wq[sl].T) * scale).astype(bf),
                "wkT": np.ascontiguousarray(Wk[sl].T).astype(bf),
                "wvT": np.ascontiguousarray(Wv[sl].T).astype(bf),
            }
        )
    return in_maps


LAST_RESULTS = None


def kernel(x, Wq, Wk, Wv, _trace=False):
    global LAST_RESULTS
    in_maps = make_in_maps(x, Wq, Wk, Wv)
    nc = build_program(T_FULL)
    res = bass_utils.run_bass_kernel_spmd(
        nc, in_maps, core_ids=list(range(N_CORES)), trace=_trace
    )
    LAST_RESULTS = res
    full = np.empty((B, T_FULL, C), np.float32)
    for core in range(N_CORES):
        b, hg = divmod(core, N_CORES // B)
        full[b, :, hg * CPC : (hg + 1) * CPC] = res.results[core]["out"]
    return full
